# revision 6
# baseline (speedup 1.0000x reference)
"""Trainium2 Bass kernel for the LSTM decoder problem (nn_Decoder).

Math (reference):
    h0 = latent @ W_fc.T + b_fc ;  c0 = 0 ;  x0 = obs_s[-1]
    for t in 0..13:
        gates = x @ W_ih.T + h @ W_hh.T + (b_ih + b_hh)      # [B, 4H], order i,f,g,o
        c = sig(f)*c + sig(i)*tanh(g)
        h = sig(o)*tanh(c)
        x = h @ W_mlp.T + b_mlp                              # [B, 39] -> output step t
Key algebraic fold: for t>=1, x_t = W_mlp h_{t-1} + b_mlp, so
    gates_t = (W_ih W_mlp + W_hh) h_{t-1} + (b_ih + b_hh + W_ih b_mlp)
i.e. the recurrence only needs h. W_combo := W_ih@W_mlp + W_hh  [4H, H].

Device layout: batch is data-parallel over 8 cores (16384 each). Per core the
shard is split into NSC=4 superchunks ("chains") of 4 groups x C=1024 batch
columns, giving four independent recurrences to interleave. Activations live
in [feature, batch-column] layout with the 4 groups stacked on the 128 SBUF
partitions (group j on partitions 32j:32j+32).

Engine assignment per (t, sc): PE computes gates via block-diag stationary
weights; ACT does the 4 gate activations + tanh(c) (the throughput bound);
DVE does the cell arithmetic in fp16 (2x mode) and the PSUM->SBUF mlp
bias-add/stage; GPSIMD computes h = sig(o)*tanh(c) (off the tanh path).

PSUM budgeting (the real pacer in earlier versions): gate tiles [128,1024]
f32 (2 banks) rotate in their own 2-buffer tag; mlp tiles [78,1024] rotate in
a SEPARATE 2-buffer tag, so the output path never steals gate buffers. The
mlp+stage for step t runs one step lagged (h is double-buffered), leaving a
full step of slack before its buffers are needed again.
"""

import numpy as np
from contextlib import ExitStack

import concourse.bass as bass
import concourse.bacc as bacc
import concourse.tile as tile
from concourse import mybir
from concourse.bass_utils import run_bass_kernel_spmd

POSE, H, LATD = 39, 32, 16
B_TOTAL, T = 131072, 14
NCORES = 8
BS = B_TOTAL // NCORES          # 16384 batch per core
NSC = 4                         # superchunks (independent chains) per core
GROUPS = 4                      # batch groups stacked on partitions
C = BS // (NSC * GROUPS)        # 1024 columns per group per superchunk
MMW = 512                       # matmul moving free dim (HW max)
NMM = C // MMW                  # 2 chunks per full-width matmul
# packed-constant column offsets (fp16 weight pack)
OW_G, OW_HH, OW_IH, OW_FC, OW_MLP = 0, 512, 1024, 1152, 1280
WPACK_COLS = 1358

F32 = mybir.dt.float32
F16 = mybir.dt.float16
SIG = mybir.ActivationFunctionType.Sigmoid
TANH = mybir.ActivationFunctionType.Tanh
MULT = mybir.AluOpType.mult
ADD = mybir.AluOpType.add

# gate emission order: g (tanh) first so sig(i)*tanh(g) can start early,
# then i, f, o.  t=0 skips f (c0 = 0).
GATE_ORDER = (2, 0, 1, 3)
GATE_ORDER_T0 = (2, 0, 3)


def _build_body(ctx, tc, io, _step_schedule=tuple(range(T))):
    nc = tc.nc

    consts = ctx.enter_context(tc.tile_pool(name="consts", bufs=1))
    xin = ctx.enter_context(tc.tile_pool(name="xin", bufs=2))
    state = ctx.enter_context(tc.tile_pool(name="state", bufs=1))
    acts = ctx.enter_context(tc.tile_pool(name="acts", bufs=3))
    tmps = ctx.enter_context(tc.tile_pool(name="tmps", bufs=3))
    stg = ctx.enter_context(tc.tile_pool(name="stg", bufs=3))
    ps = ctx.enter_context(tc.tile_pool(name="ps", bufs=2, space="PSUM"))

    # ---- constants to SBUF (packed: 2 DMAs keep dependency fan-in small) ----
    wpack_sb = consts.tile([128, WPACK_COLS], F16, tag="wpack", name="wpack")
    bpack_sb = consts.tile([128, 10], F32, tag="bpack", name="bpack")
    nc.sync.dma_start(out=wpack_sb, in_=io["wpack"])
    nc.sync.dma_start(out=bpack_sb, in_=io["bpack"])
    wg_sb = [wpack_sb[:, OW_G + 128 * g : OW_G + 128 * (g + 1)] for g in range(4)]
    whh_sb = [wpack_sb[:, OW_HH + 128 * g : OW_HH + 128 * (g + 1)] for g in range(4)]
    wih_sb = [wpack_sb[0:POSE, OW_IH + H * g : OW_IH + H * (g + 1)] for g in range(4)]
    wfc_sb = wpack_sb[0:64, OW_FC : OW_FC + 128]
    wmlp_sb = wpack_sb[:, OW_MLP : OW_MLP + 78]
    bg0_sb = bpack_sb[:, 0:4]
    bgc_sb = bpack_sb[:, 4:8]
    bfc_sb = bpack_sb[:, 8:9]
    bmlp_sb = bpack_sb[0:78, 9:10]

    # x0 viewed as [NSC, 39, GROUPS, C] so per-sc chunks DMA cleanly
    x0re = io["x0"].rearrange("p (s g c) -> s p g c", s=NSC, g=GROUPS)

    # ---- per-superchunk persistent state (fp16) ----
    # h double-buffered: step t's gates read hb[sc][t%2], the cell writes
    # hb[sc][(t+1)%2]; the lagged mlp for step t-1 reads hb[sc][t%2].
    hb = [
        [state.tile([128, C], F16, tag=f"h{sc}_{p}", name=f"h{sc}_{p}") for p in range(2)]
        for sc in range(NSC)
    ]
    cst = [state.tile([128, C], F16, tag=f"c{sc}", name=f"c{sc}") for sc in range(NSC)]
    x0c = [None] * NSC

    # ---- h0 = W_fc @ latent + b_fc (block-diag over 4 stacked groups) ----
    for sc in range(NSC):
        lat_sb = xin.tile([64, C], F16, tag="lat", name="lat", bufs=NSC)
        nc.sync.dma_start(out=lat_sb, in_=io["lat"][sc])
        p0 = ps.tile([128, C], F32, tag="psg", name="ps0", bufs=3)
        for m in range(NMM):
            nc.tensor.matmul(
                p0[:, m * MMW : (m + 1) * MMW],
                lhsT=wfc_sb,
                rhs=lat_sb[:, m * MMW : (m + 1) * MMW],
                start=True,
                stop=True,
            )
        nc.vector.tensor_tensor(hb[sc][0], p0, bfc_sb.to_broadcast((128, C)), ADD)
        # prefetch x0 for the t=0 gate matmuls
        x0c[sc] = xin.tile([POSE, GROUPS, C], F16, tag="x0", name="x0", bufs=NSC)
        nc.sync.dma_start(out=x0c[sc], in_=x0re[sc])

    def emit_mlp(t, sc):
        hB = hb[sc][(t + 1) % 2]  # h after step t's cell update
        for pr in range(2):
            st = stg.tile([78, C], F16, tag=f"s{pr}", name=f"s{pr}")
            for m in range(NMM):
                mc = slice(m * MMW, (m + 1) * MMW)
                # [78,512] psum tiles (1 bank) keep the mlp tag at 2 banks
                # total, freeing 6 banks for a deeper gate rotation
                pm = ps.tile([78, MMW], F32, tag="psm", name="psm", bufs=2)
                nc.tensor.matmul(
                    pm,
                    lhsT=wmlp_sb[64 * pr : 64 * (pr + 1), :],
                    rhs=hB[64 * pr : 64 * (pr + 1), mc],
                    start=True,
                    stop=True,
                )
                nc.vector.tensor_tensor(
                    st[:, mc], pm, bmlp_sb.to_broadcast((78, MMW)), ADD
                )
            nc.sync.dma_start(out=io["out"][t, sc, pr], in_=st)

    # ---- decode steps ----
    for t in _step_schedule:
        # phase 1: gates + cell update for each chain
        for sc in range(NSC):
            hA = hb[sc][t % 2]
            hN = hb[sc][(t + 1) % 2]
            sig = {}
            for g in GATE_ORDER_T0 if t == 0 else GATE_ORDER:
                pg = ps.tile([128, C], F32, tag="psg", name="psg", bufs=3)
                for m in range(NMM):
                    mo = pg[:, m * MMW : (m + 1) * MMW]
                    mc = slice(m * MMW, (m + 1) * MMW)
                    if t == 0:
                        # full-width h-part first: start=True sets has_written
                        # on all partitions; x-part matmuls purely accumulate.
                        nc.tensor.matmul(
                            mo, lhsT=whh_sb[g], rhs=hA[:, mc],
                            start=True, stop=False,
                        )
                        for j in range(GROUPS):
                            nc.tensor.matmul(
                                pg[32 * j : 32 * (j + 1), m * MMW : (m + 1) * MMW],
                                lhsT=wih_sb[g],
                                rhs=x0c[sc][:, j, mc],
                                start=False,
                                stop=(j == GROUPS - 1),
                                tile_position=(0, 32 * j),
                            )
                    else:
                        nc.tensor.matmul(
                            mo, lhsT=wg_sb[g], rhs=hA[:, mc],
                            start=True, stop=True,
                        )
                a = acts.tile([128, C], F16, tag=f"a{g}", name=f"a{g}")
                bias = (bg0_sb if t == 0 else bgc_sb)[:, g : g + 1]
                nc.scalar.activation(a, pg, TANH if g == 2 else SIG, bias=bias)
                sig[g] = a
            # LSTM cell update (fp16 on DVE, 2x mode)
            if t == 0:
                # c0 = 0 -> c1 = sig(i) * tanh(g)
                nc.vector.tensor_tensor(cst[sc], sig[0], sig[2], MULT)
            else:
                t2 = tmps.tile([128, C], F16, tag="t2", name="t2")
                nc.vector.tensor_tensor(t2, sig[0], sig[2], MULT)
                t1 = tmps.tile([128, C], F16, tag="t1", name="t1")
                nc.vector.tensor_tensor(t1, sig[1], cst[sc], MULT)
                nc.vector.tensor_tensor(cst[sc], t1, t2, ADD)
            tct = acts.tile([128, C], F16, tag="tc", name="tc")
            nc.scalar.activation(tct, cst[sc], TANH)
            # h off the tanh path on gpsimd (consumers run >2us later)
            nc.gpsimd.tensor_tensor(hN, sig[3], tct, MULT)
        # phase 2: mlp + stage + out-DMA for the PREVIOUS step (one-step
        # software pipeline; its PSUM tag is private so it never steals
        # gate buffers)
        if t > 0:
            for sc in range(NSC):
                emit_mlp(t - 1, sc)
    # epilogue: flush the final step's output
    for sc in range(NSC):
        emit_mlp(T - 1, sc)


_NC_CACHE = {}


def build_nc(mode="real"):
    global _NC_CACHE
    if mode in _NC_CACHE:
        return _NC_CACHE[mode]
    nc = bacc.Bacc("TRN2", target_bir_lowering=False, debug=False)
    io = {
        "x0": nc.dram_tensor("x0", [POSE, BS], F16, kind="ExternalInput").ap(),
        "lat": nc.dram_tensor("lat", [NSC, 64, C], F16, kind="ExternalInput").ap(),
        "wpack": nc.dram_tensor("wpack", [128, WPACK_COLS], F16, kind="ExternalInput").ap(),
        "bpack": nc.dram_tensor("bpack", [128, 10], F32, kind="ExternalInput").ap(),
        "out": nc.dram_tensor("out", [T, NSC, 2, 78, C], F16, kind="ExternalOutput").ap(),
    }
    with tile.TileContext(nc) as tc:
        with ExitStack() as ctx:
            _build_body(ctx, tc, io)
    nc.compile()
    _NC_CACHE[mode] = nc
    return nc


def prep_inputs(obs_s, latent, W_ih, W_hh, b_ih, b_hh, W_fc, b_fc, W_mlp, b_mlp):
    """Host-side weight folding + sharding. Returns per-core input maps."""
    f32, f16 = np.float32, np.float16
    W_ih = np.asarray(W_ih, f32)
    W_hh = np.asarray(W_hh, f32)
    b_ih = np.asarray(b_ih, f32)
    b_hh = np.asarray(b_hh, f32)
    W_fc = np.asarray(W_fc, f32)
    b_fc = np.asarray(b_fc, f32)
    W_mlp = np.asarray(W_mlp, f32)
    b_mlp = np.asarray(b_mlp, f32)

    W_combo = W_ih @ W_mlp + W_hh                    # [4H, H]
    b_combo = b_ih + b_hh + W_ih @ b_mlp             # [4H]

    wg = np.zeros((4, 128, 128), f32)
    whh_bd = np.zeros((4, 128, 128), f32)
    for g in range(4):
        for j in range(4):
            wg[g, 32 * j : 32 * (j + 1), 32 * j : 32 * (j + 1)] = W_combo[
                32 * g : 32 * (g + 1)
            ].T
            whh_bd[g, 32 * j : 32 * (j + 1), 32 * j : 32 * (j + 1)] = W_hh[
                32 * g : 32 * (g + 1)
            ].T
    wih_t = np.stack([W_ih[32 * g : 32 * (g + 1)].T for g in range(4)])  # [4,39,32]
    wfc_bd = np.zeros((64, 128), f32)
    for j in range(4):
        wfc_bd[16 * j : 16 * (j + 1), 32 * j : 32 * (j + 1)] = W_fc.T
    wmlp = np.zeros((128, 78), f32)
    for half in range(2):
        for j in range(2):
            wmlp[
                64 * half + 32 * j : 64 * half + 32 * (j + 1),
                39 * j : 39 * (j + 1),
            ] = W_mlp.T
    bg0 = np.stack(
        [np.tile(b_ih[32 * g : 32 * (g + 1)] + b_hh[32 * g : 32 * (g + 1)], 4) for g in range(4)]
    )[..., None].astype(f32)
    bgc = np.stack([np.tile(b_combo[32 * g : 32 * (g + 1)], 4) for g in range(4)])[
        ..., None
    ].astype(f32)
    bfc_v = np.tile(b_fc, 4)[:, None].astype(f32)
    bmlp_v = np.tile(b_mlp, 2)[:, None].astype(f32)

    x0T = np.ascontiguousarray(np.asarray(obs_s[-1], f32).T).astype(f16)  # [39, B]
    latT = np.ascontiguousarray(np.asarray(latent, f32).T).astype(f16)    # [16, B]

    wpack = np.zeros((128, WPACK_COLS), f32)
    for g in range(4):
        wpack[:, OW_G + 128 * g : OW_G + 128 * (g + 1)] = wg[g]
        wpack[:, OW_HH + 128 * g : OW_HH + 128 * (g + 1)] = whh_bd[g]
        wpack[: POSE, OW_IH + H * g : OW_IH + H * (g + 1)] = wih_t[g]
    wpack[:64, OW_FC : OW_FC + 128] = wfc_bd
    wpack[:, OW_MLP : OW_MLP + 78] = wmlp
    bpack = np.zeros((128, 10), f32)
    bpack[:, 0:4] = bg0[..., 0].T
    bpack[:, 4:8] = bgc[..., 0].T
    bpack[:, 8] = bfc_v[:, 0]
    bpack[:78, 9] = bmlp_v[:, 0]
    common = {"wpack": wpack.astype(f16), "bpack": bpack}
    in_maps = []
    for c in range(NCORES):
        base = c * BS
        lp = np.empty((NSC, 64, C), f16)
        for sc in range(NSC):
            for j in range(GROUPS):
                s = base + sc * GROUPS * C + j * C
                lp[sc, 16 * j : 16 * (j + 1), :] = latT[:, s : s + C]
        m = dict(common)
        m["x0"] = np.ascontiguousarray(x0T[:, base : base + BS])
        m["lat"] = lp
        in_maps.append(m)
    return in_maps


def assemble_output(per_core_out):
    """per_core_out: list of [T, NSC, 2, 78, C] (fp16) arrays -> [T, B, 39] f32."""
    preds = np.empty((T, B_TOTAL, POSE), np.float32)
    for c in range(NCORES):
        arr = np.asarray(per_core_out[c], np.float32)
        a = (
            arr.reshape(T, NSC, 2, 2, POSE, C)
            .transpose(0, 1, 2, 3, 5, 4)
            .reshape(T, BS, POSE)
        )
        preds[:, c * BS : (c + 1) * BS] = a
    return preds


def kernel(obs_s, latent, W_ih, W_hh, b_ih, b_hh, W_fc, b_fc, W_mlp, b_mlp, pred_len):
    assert int(pred_len) == T, f"kernel hardcodes pred_len={T}, got {pred_len}"
    in_maps = prep_inputs(
        obs_s, latent, W_ih, W_hh, b_ih, b_hh, W_fc, b_fc, W_mlp, b_mlp
    )
    nc = build_nc()
    res = run_bass_kernel_spmd(nc, in_maps, core_ids=list(range(NCORES)))
    return assemble_output([res.results[c]["out"] for c in range(NCORES)])


# revision 7
# speedup vs baseline: 1.4061x; 1.4061x over previous
"""Trainium2 Bass kernel for the LSTM decoder problem (nn_Decoder).

Math (reference):
    h0 = latent @ W_fc.T + b_fc ;  c0 = 0 ;  x0 = obs_s[-1]
    for t in 0..13:
        gates = x @ W_ih.T + h @ W_hh.T + (b_ih + b_hh)      # [B, 4H], order i,f,g,o
        c = sig(f)*c + sig(i)*tanh(g)
        h = sig(o)*tanh(c)
        x = h @ W_mlp.T + b_mlp                              # [B, 39] -> output step t
Key algebraic fold: for t>=1, x_t = W_mlp h_{t-1} + b_mlp, so
    gates_t = (W_ih W_mlp + W_hh) h_{t-1} + (b_ih + b_hh + W_ih b_mlp)
i.e. the recurrence only needs h. W_combo := W_ih@W_mlp + W_hh  [4H, H].

Device layout: batch is data-parallel over 8 cores (16384 each). Per core the
shard is split into NSC=4 superchunks ("chains") of 4 groups x C=1024 batch
columns, giving four independent recurrences to interleave. Activations live
in [feature, batch-column] layout with the 4 groups stacked on the 128 SBUF
partitions (group j on partitions 32j:32j+32).

Engine assignment per (t, sc): PE computes gates via block-diag stationary
weights; ACT does the 4 gate activations + tanh(c) (the throughput bound);
DVE does the cell arithmetic in fp16 (2x mode) and the PSUM->SBUF mlp
bias-add/stage; GPSIMD computes h = sig(o)*tanh(c) (off the tanh path).

PSUM budgeting (the real pacer in earlier versions): gate tiles [128,1024]
f32 (2 banks) rotate in their own 2-buffer tag; mlp tiles [78,1024] rotate in
a SEPARATE 2-buffer tag, so the output path never steals gate buffers. The
mlp+stage for step t runs one step lagged (h is double-buffered), leaving a
full step of slack before its buffers are needed again.
"""

import numpy as np
from contextlib import ExitStack

import concourse.bass as bass
import concourse.bacc as bacc
import concourse.tile as tile
from concourse import mybir
from concourse.bass_utils import run_bass_kernel_spmd

POSE, H, LATD = 39, 32, 16
B_TOTAL, T = 131072, 14
NCORES = 8
BS = B_TOTAL // NCORES          # 16384 batch per core
NSC = 4                         # superchunks (independent chains) per core
GROUPS = 4                      # batch groups stacked on partitions
C = BS // (NSC * GROUPS)        # 1024 columns per group per superchunk
MMW = 512                       # matmul moving free dim (HW max)
NMM = C // MMW                  # 2 chunks per full-width matmul
# packed-constant column offsets (fp16 weight pack)
OW_G, OW_HH, OW_IH, OW_FC, OW_MLP = 0, 512, 1024, 1152, 1280
WPACK_COLS = 1358

F32 = mybir.dt.float32
F16 = mybir.dt.float16
SIG = mybir.ActivationFunctionType.Sigmoid
TANH = mybir.ActivationFunctionType.Tanh
MULT = mybir.AluOpType.mult
ADD = mybir.AluOpType.add

# gate emission order: g (tanh) first so sig(i)*tanh(g) can start early,
# then i, f, o.  t=0 skips f (c0 = 0).
GATE_ORDER = (2, 0, 1, 3)
GATE_ORDER_T0 = (2, 0, 3)


def _build_body(ctx, tc, io, _step_schedule=tuple(range(T))):
    nc = tc.nc

    consts = ctx.enter_context(tc.tile_pool(name="consts", bufs=1))
    xin = ctx.enter_context(tc.tile_pool(name="xin", bufs=2))
    state = ctx.enter_context(tc.tile_pool(name="state", bufs=1))
    acts = ctx.enter_context(tc.tile_pool(name="acts", bufs=3))
    tmps = ctx.enter_context(tc.tile_pool(name="tmps", bufs=3))
    stg = ctx.enter_context(tc.tile_pool(name="stg", bufs=3))
    ps = ctx.enter_context(tc.tile_pool(name="ps", bufs=2, space="PSUM"))

    # ---- constants to SBUF (packed: 2 DMAs keep dependency fan-in small) ----
    wpack_sb = consts.tile([128, WPACK_COLS], F16, tag="wpack", name="wpack")
    bpack_sb = consts.tile([128, 10], F32, tag="bpack", name="bpack")
    nc.sync.dma_start(out=wpack_sb, in_=io["wpack"])
    nc.sync.dma_start(out=bpack_sb, in_=io["bpack"])
    wg_sb = [wpack_sb[:, OW_G + 128 * g : OW_G + 128 * (g + 1)] for g in range(4)]
    whh_sb = [wpack_sb[:, OW_HH + 128 * g : OW_HH + 128 * (g + 1)] for g in range(4)]
    wih_sb = [wpack_sb[0:POSE, OW_IH + H * g : OW_IH + H * (g + 1)] for g in range(4)]
    wfc_sb = wpack_sb[0:64, OW_FC : OW_FC + 128]
    wmlp_sb = wpack_sb[:, OW_MLP : OW_MLP + 78]
    bg0_sb = bpack_sb[:, 0:4]
    bgc_sb = bpack_sb[:, 4:8]
    bfc_sb = bpack_sb[:, 8:9]
    bmlp_sb = bpack_sb[0:78, 9:10]

    # x0 viewed as [NSC, 39, GROUPS, C] so per-sc chunks DMA cleanly
    x0re = io["x0"].rearrange("p (s g c) -> s p g c", s=NSC, g=GROUPS)

    # ---- per-superchunk persistent state (fp16) ----
    # h double-buffered: step t's gates read hb[sc][t%2], the cell writes
    # hb[sc][(t+1)%2]; the lagged mlp for step t-1 reads hb[sc][t%2].
    hb = [
        [state.tile([128, C], F16, tag=f"h{sc}_{p}", name=f"h{sc}_{p}") for p in range(2)]
        for sc in range(NSC)
    ]
    cst = [state.tile([128, C], F16, tag=f"c{sc}", name=f"c{sc}") for sc in range(NSC)]
    x0c = [None] * NSC

    # ---- h0 = W_fc @ latent + b_fc (block-diag over 4 stacked groups) ----
    for sc in range(NSC):
        lat_sb = xin.tile([64, C], F16, tag="lat", name="lat", bufs=NSC)
        nc.sync.dma_start(out=lat_sb, in_=io["lat"][sc])
        p0 = ps.tile([128, C], F32, tag="psg", name="ps0", bufs=2)
        for m in range(NMM):
            nc.tensor.matmul(
                p0[:, m * MMW : (m + 1) * MMW],
                lhsT=wfc_sb,
                rhs=lat_sb[:, m * MMW : (m + 1) * MMW],
                start=True,
                stop=True,
            )
        nc.vector.tensor_tensor(hb[sc][0], p0, bfc_sb.to_broadcast((128, C)), ADD)
        # prefetch x0 for the t=0 gate matmuls
        x0c[sc] = xin.tile([POSE, GROUPS, C], F16, tag="x0", name="x0", bufs=NSC)
        nc.sync.dma_start(out=x0c[sc], in_=x0re[sc])

    def emit_mlp(t, sc):
        hB = hb[sc][(t + 1) % 2]  # h after step t's cell update
        for pr in range(2):
            pm = ps.tile([78, C], F32, tag="psm", name="psm", bufs=2)
            for m in range(NMM):
                mc = slice(m * MMW, (m + 1) * MMW)
                nc.tensor.matmul(
                    pm[:, mc],
                    lhsT=wmlp_sb[64 * pr : 64 * (pr + 1), :],
                    rhs=hB[64 * pr : 64 * (pr + 1), mc],
                    start=True,
                    stop=True,
                )
            st = stg.tile([78, C], F16, tag=f"s{pr}", name=f"s{pr}")
            nc.vector.tensor_tensor(
                st, pm, bmlp_sb.to_broadcast((78, C)), ADD
            )
            nc.sync.dma_start(out=io["out"][t, sc, pr], in_=st)

    # ---- decode steps ----
    for t in _step_schedule:
        # phase 1: gates + cell update for each chain
        for sc in range(NSC):
            hA = hb[sc][t % 2]
            hN = hb[sc][(t + 1) % 2]
            sig = {}
            for g in GATE_ORDER_T0 if t == 0 else GATE_ORDER:
                pg = ps.tile([128, C], F32, tag="psg", name="psg", bufs=2)
                for m in range(NMM):
                    mo = pg[:, m * MMW : (m + 1) * MMW]
                    mc = slice(m * MMW, (m + 1) * MMW)
                    if t == 0:
                        # full-width h-part first: start=True sets has_written
                        # on all partitions; x-part matmuls purely accumulate.
                        nc.tensor.matmul(
                            mo, lhsT=whh_sb[g], rhs=hA[:, mc],
                            start=True, stop=False,
                        )
                        for j in range(GROUPS):
                            nc.tensor.matmul(
                                pg[32 * j : 32 * (j + 1), m * MMW : (m + 1) * MMW],
                                lhsT=wih_sb[g],
                                rhs=x0c[sc][:, j, mc],
                                start=False,
                                stop=(j == GROUPS - 1),
                                tile_position=(0, 32 * j),
                            )
                    else:
                        nc.tensor.matmul(
                            mo, lhsT=wg_sb[g], rhs=hA[:, mc],
                            start=True, stop=True,
                        )
                a = acts.tile([128, C], F16, tag=f"a{g}", name=f"a{g}")
                bias = (bg0_sb if t == 0 else bgc_sb)[:, g : g + 1]
                nc.scalar.activation(a, pg, TANH if g == 2 else SIG, bias=bias)
                sig[g] = a
            # LSTM cell update (fp16 on DVE, 2x mode)
            if t == 0:
                # c0 = 0 -> c1 = sig(i) * tanh(g)
                nc.vector.tensor_tensor(cst[sc], sig[0], sig[2], MULT)
            else:
                t2 = tmps.tile([128, C], F16, tag="t2", name="t2")
                nc.vector.tensor_tensor(t2, sig[0], sig[2], MULT)
                t1 = tmps.tile([128, C], F16, tag="t1", name="t1")
                nc.vector.tensor_tensor(t1, sig[1], cst[sc], MULT)
                nc.vector.tensor_tensor(cst[sc], t1, t2, ADD)
            tct = acts.tile([128, C], F16, tag="tc", name="tc")
            nc.scalar.activation(tct, cst[sc], TANH)
            nc.vector.tensor_tensor(hN, sig[3], tct, MULT)
        # phase 2: mlp + stage + out-DMA for the PREVIOUS step (one-step
        # software pipeline; its PSUM tag is private so it never steals
        # gate buffers)
        if t > 0:
            for sc in range(NSC):
                emit_mlp(t - 1, sc)
    # epilogue: flush the final step's output
    for sc in range(NSC):
        emit_mlp(T - 1, sc)


_NC_CACHE = {}


def build_nc(mode="real"):
    global _NC_CACHE
    if mode in _NC_CACHE:
        return _NC_CACHE[mode]
    nc = bacc.Bacc("TRN2", target_bir_lowering=False, debug=False)
    io = {
        "x0": nc.dram_tensor("x0", [POSE, BS], F16, kind="ExternalInput").ap(),
        "lat": nc.dram_tensor("lat", [NSC, 64, C], F16, kind="ExternalInput").ap(),
        "wpack": nc.dram_tensor("wpack", [128, WPACK_COLS], F16, kind="ExternalInput").ap(),
        "bpack": nc.dram_tensor("bpack", [128, 10], F32, kind="ExternalInput").ap(),
        "out": nc.dram_tensor("out", [T, NSC, 2, 78, C], F16, kind="ExternalOutput").ap(),
    }
    with tile.TileContext(nc) as tc:
        with ExitStack() as ctx:
            _build_body(ctx, tc, io)
    nc.compile()
    _NC_CACHE[mode] = nc
    return nc


def prep_inputs(obs_s, latent, W_ih, W_hh, b_ih, b_hh, W_fc, b_fc, W_mlp, b_mlp):
    """Host-side weight folding + sharding. Returns per-core input maps."""
    f32, f16 = np.float32, np.float16
    W_ih = np.asarray(W_ih, f32)
    W_hh = np.asarray(W_hh, f32)
    b_ih = np.asarray(b_ih, f32)
    b_hh = np.asarray(b_hh, f32)
    W_fc = np.asarray(W_fc, f32)
    b_fc = np.asarray(b_fc, f32)
    W_mlp = np.asarray(W_mlp, f32)
    b_mlp = np.asarray(b_mlp, f32)

    W_combo = W_ih @ W_mlp + W_hh                    # [4H, H]
    b_combo = b_ih + b_hh + W_ih @ b_mlp             # [4H]

    wg = np.zeros((4, 128, 128), f32)
    whh_bd = np.zeros((4, 128, 128), f32)
    for g in range(4):
        for j in range(4):
            wg[g, 32 * j : 32 * (j + 1), 32 * j : 32 * (j + 1)] = W_combo[
                32 * g : 32 * (g + 1)
            ].T
            whh_bd[g, 32 * j : 32 * (j + 1), 32 * j : 32 * (j + 1)] = W_hh[
                32 * g : 32 * (g + 1)
            ].T
    wih_t = np.stack([W_ih[32 * g : 32 * (g + 1)].T for g in range(4)])  # [4,39,32]
    wfc_bd = np.zeros((64, 128), f32)
    for j in range(4):
        wfc_bd[16 * j : 16 * (j + 1), 32 * j : 32 * (j + 1)] = W_fc.T
    wmlp = np.zeros((128, 78), f32)
    for half in range(2):
        for j in range(2):
            wmlp[
                64 * half + 32 * j : 64 * half + 32 * (j + 1),
                39 * j : 39 * (j + 1),
            ] = W_mlp.T
    bg0 = np.stack(
        [np.tile(b_ih[32 * g : 32 * (g + 1)] + b_hh[32 * g : 32 * (g + 1)], 4) for g in range(4)]
    )[..., None].astype(f32)
    bgc = np.stack([np.tile(b_combo[32 * g : 32 * (g + 1)], 4) for g in range(4)])[
        ..., None
    ].astype(f32)
    bfc_v = np.tile(b_fc, 4)[:, None].astype(f32)
    bmlp_v = np.tile(b_mlp, 2)[:, None].astype(f32)

    x0T = np.ascontiguousarray(np.asarray(obs_s[-1], f32).T).astype(f16)  # [39, B]
    latT = np.ascontiguousarray(np.asarray(latent, f32).T).astype(f16)    # [16, B]

    wpack = np.zeros((128, WPACK_COLS), f32)
    for g in range(4):
        wpack[:, OW_G + 128 * g : OW_G + 128 * (g + 1)] = wg[g]
        wpack[:, OW_HH + 128 * g : OW_HH + 128 * (g + 1)] = whh_bd[g]
        wpack[: POSE, OW_IH + H * g : OW_IH + H * (g + 1)] = wih_t[g]
    wpack[:64, OW_FC : OW_FC + 128] = wfc_bd
    wpack[:, OW_MLP : OW_MLP + 78] = wmlp
    bpack = np.zeros((128, 10), f32)
    bpack[:, 0:4] = bg0[..., 0].T
    bpack[:, 4:8] = bgc[..., 0].T
    bpack[:, 8] = bfc_v[:, 0]
    bpack[:78, 9] = bmlp_v[:, 0]
    common = {"wpack": wpack.astype(f16), "bpack": bpack}
    in_maps = []
    for c in range(NCORES):
        base = c * BS
        lp = np.empty((NSC, 64, C), f16)
        for sc in range(NSC):
            for j in range(GROUPS):
                s = base + sc * GROUPS * C + j * C
                lp[sc, 16 * j : 16 * (j + 1), :] = latT[:, s : s + C]
        m = dict(common)
        m["x0"] = np.ascontiguousarray(x0T[:, base : base + BS])
        m["lat"] = lp
        in_maps.append(m)
    return in_maps


def assemble_output(per_core_out):
    """per_core_out: list of [T, NSC, 2, 78, C] (fp16) arrays -> [T, B, 39] f32."""
    preds = np.empty((T, B_TOTAL, POSE), np.float32)
    for c in range(NCORES):
        arr = np.asarray(per_core_out[c], np.float32)
        a = (
            arr.reshape(T, NSC, 2, 2, POSE, C)
            .transpose(0, 1, 2, 3, 5, 4)
            .reshape(T, BS, POSE)
        )
        preds[:, c * BS : (c + 1) * BS] = a
    return preds


def kernel(obs_s, latent, W_ih, W_hh, b_ih, b_hh, W_fc, b_fc, W_mlp, b_mlp, pred_len):
    assert int(pred_len) == T, f"kernel hardcodes pred_len={T}, got {pred_len}"
    in_maps = prep_inputs(
        obs_s, latent, W_ih, W_hh, b_ih, b_hh, W_fc, b_fc, W_mlp, b_mlp
    )
    nc = build_nc()
    res = run_bass_kernel_spmd(nc, in_maps, core_ids=list(range(NCORES)))
    return assemble_output([res.results[c]["out"] for c in range(NCORES)])
